# revision 3
# baseline (speedup 1.0000x reference)
"""Chamfer loss (p3 variant) on 8 Trainium2 NeuronCores.

Computes, for p, q of shape (2, 64, 1024, 4) fp32:
    d2[c,b,n,m] = ||p3[c,b,n] - q3[c,b,m]||^2   (p3 = spatial comps 1:4)
    loss = sum(min_m sqrt(max(d2,0)+1e-12)) + sum(min_n sqrt(...))

Strategy (data-parallel over batch, 8 batches per core):
  - e[n,m] = p3.q3' - 0.5|p3|^2 - 0.5|q3'|^2 = -d2/2, produced directly in
    PSUM by a K=5 matmul over the embedding rows
       lhsT = [x, y, z, -0.5*nrm, 1],  rhs = [x', y', z', 1, -0.5*nrm'].
  - row-min of d2 == -2 * row-max of e. A custom fused DVE op
    (MAXPAIR_REDUCE: out = max(in0,in1), accum = max-reduce) consumes two
    512-wide PSUM/SBUF halves per instruction, so every d2 element crosses
    the DVE at 2 elems/cycle. ScalarE copies one half PSUM->SBUF to enable
    the dual-port read.
  - both passes (p-major row-min and q-major col-min) run as independent
    matmul phases; sqrt (+2 Heron refinements) and the final sum happen on
    a [128, 256] tile of per-chunk minima.
"""

import sys

sys.path.insert(0, "/opt/trn_rl_repo")

from contextlib import ExitStack

import numpy as np

import concourse.bass as bass
import concourse.tile as tile
from concourse import bacc, mybir

# --------------------------------------------------------------------------
# Custom DVE op: out = max(in0, in1); accum_out = max(s0, max_k out[:, k])
# Registered by appending to concourse.dve_ops.OPS (see
# trainium-docs/custom-instructions/04-custom-dve-api.md).
# --------------------------------------------------------------------------
import concourse.dve_ops as dve_ops
from concourse.dve_ops import DveOp
from concourse.dve_spec import C0, Spec, Src0, Src1, lower as dve_lower, maxx
from concourse.dve_uop import DveOpSpec


def _ref_maxpair_reduce(in0, in1, c0, c1, c2):
    b = np.maximum(in0.astype(np.float32), in1.astype(np.float32))
    P = b.shape[0]
    acc = np.maximum(
        np.broadcast_to(np.asarray(c0, np.float32), (P, 1)),
        b.reshape(P, -1).max(axis=-1, keepdims=True),
    ).astype(np.float32)
    return b, acc


def _register_maxpair():
    spec = Spec(
        body=maxx(Src0, Src1),
        accum=maxx,
        accum_init=C0,
        reference=_ref_maxpair_reduce,
    )
    shas = {}
    for ver in ("v3", "v4"):
        uops = dve_lower(spec, ver=ver)
        shas[ver] = DveOpSpec(
            name="MAXPAIR_REDUCE", opcode=0, uops=uops, rd1_en=True
        ).sha(ver)
    op = DveOp("MAXPAIR_REDUCE", spec, subdim=False, uops_sha=shas)
    if all(o.name != op.name for o in dve_ops.OPS):
        dve_ops.OPS.append(op)
        dve_ops.CUSTOM_DVE_SPECS[op.name] = spec
        dve_ops._SUB_OPCODE_FOR_NAME[op.name] = (
            max(dve_ops._SUB_OPCODE_FOR_NAME.values()) + 1
        )
        assert dve_ops._SUB_OPCODE_FOR_NAME[op.name] < 0x20
    return op


MAXPAIR_REDUCE = _register_maxpair()

# --------------------------------------------------------------------------
# Kernel build
# --------------------------------------------------------------------------
N_CORES = 8
CH = 2  # complex channels
BPC = 8  # batches per core (64 / 8 cores)
N = 1024  # points per set
NCHUNK = N // 128  # partition chunks per batch
F32 = mybir.dt.float32
NEG_SEED = -3.0e38
AX = mybir.AxisListType
ALU = mybir.AluOpType


def build_kernel(nc, repeat=1):
    p_ap = nc.dram_tensor("p", [CH, BPC, N, 4], F32, kind="ExternalInput").ap()
    q_ap = nc.dram_tensor("q", [CH, BPC, N, 4], F32, kind="ExternalInput").ap()
    out_ap = nc.dram_tensor("out", [1, 1], F32, kind="ExternalOutput").ap()
    inp = [p_ap, q_ap]

    with tile.TileContext(nc) as tc:
        with ExitStack() as ctx:
            dramp = ctx.enter_context(tc.tile_pool(name="dram", bufs=1, space="DRAM"))
            nat = ctx.enter_context(tc.tile_pool(name="nat", bufs=2))
            nrm = ctx.enter_context(tc.tile_pool(name="nrm", bufs=2))
            emb = ctx.enter_context(tc.tile_pool(name="emb", bufs=2))
            psp = ctx.enter_context(tc.tile_pool(name="psp", bufs=6, space="PSUM"))
            in1p = ctx.enter_context(tc.tile_pool(name="in1p", bufs=4))
            scr = ctx.enter_context(tc.tile_pool(name="scr", bufs=3))
            fin = ctx.enter_context(tc.tile_pool(name="fin", bufs=1))
            pssp = ctx.enter_context(tc.tile_pool(name="pssp", bufs=1, space="PSUM"))

            def body(_iv=None):
                # ---- norm rows: -0.5*|x|^2 per point, staged to DRAM in
                # flat-n order so they can be DMA'd into embedding row 3/4.
                srows = {}
                for s in range(2):
                    for c in range(CH):
                        pn = nat.tile([128, 256], F32, tag="pn")
                        nc.sync.dma_start(
                            pn[:],
                            inp[s][c].rearrange("b (x u) k -> (b x) (u k)", x=16),
                        )
                        sq = nat.tile([128, 256], F32, tag="sq")
                        nc.scalar.square(sq[:], pn[:])
                        nr = nrm.tile([128, 64], F32, tag="nr")
                        nc.vector.reduce_sum(
                            nr[:],
                            sq[:].rearrange("p (u k) -> p u k", k=4)[:, :, 1:4],
                            axis=AX.X,
                        )
                        nc.vector.tensor_scalar_mul(nr[:], nr[:], -0.5)
                        srow = dramp.tile([128, 64], F32, tag=f"srow{s}{c}")
                        nc.sync.dma_start(srow[:], nr[:])
                        srows[(s, c)] = srow

                # ---- accumulator of per-chunk maxima of e = -d2/2
                racc = fin.tile([128, 4 * BPC * NCHUNK], F32, tag="racc")

                # ones row staged at partition 0 (DVE ops cannot start at
                # partition 3/4; DMA can write there)
                ones_row = fin.tile([1, BPC * N], F32, tag="ones_row")
                nc.vector.memset(ones_row[:], 1.0)

                col = 0
                for pass_ in range(2):
                    ls, rs = (0, 1) if pass_ == 0 else (1, 0)
                    for c in range(CH):
                        L = emb.tile([5, BPC * N], F32, tag="L")
                        R = emb.tile([5, BPC * N], F32, tag="R")
                        for k in range(3):
                            nc.sync.dma_start(
                                L[k : k + 1, :],
                                inp[ls][c][:, :, k + 1].rearrange("b n -> (b n)")[
                                    None, :
                                ],
                            )
                            nc.sync.dma_start(
                                R[k : k + 1, :],
                                inp[rs][c][:, :, k + 1].rearrange("b n -> (b n)")[
                                    None, :
                                ],
                            )
                        nc.sync.dma_start(L[4:5, :], ones_row[:])
                        nc.sync.dma_start(R[3:4, :], ones_row[:])
                        nc.sync.dma_start(
                            L[3:4, :],
                            srows[(ls, c)][:].rearrange("p u -> (p u)")[None, :],
                        )
                        nc.sync.dma_start(
                            R[4:5, :],
                            srows[(rs, c)][:].rearrange("p u -> (p u)")[None, :],
                        )
                        for b in range(BPC):
                            for i in range(NCHUNK):
                                lsl = L[:, b * N + i * 128 : b * N + (i + 1) * 128]
                                ps0 = psp.tile([128, 512], F32, tag="ps")
                                nc.tensor.matmul(
                                    ps0[:],
                                    lsl,
                                    R[:, b * N : b * N + 512],
                                    start=True,
                                    stop=True,
                                )
                                ps1 = psp.tile([128, 512], F32, tag="ps")
                                nc.tensor.matmul(
                                    ps1[:],
                                    lsl,
                                    R[:, b * N + 512 : b * N + 1024],
                                    start=True,
                                    stop=True,
                                )
                                buf1 = in1p.tile([128, 512], F32, tag="b1")
                                nc.scalar.copy(buf1[:], ps1[:])
                                sc = scr.tile([128, 512], F32, tag="sc")
                                nc.vector._custom_dve(
                                    MAXPAIR_REDUCE,
                                    out=sc[:],
                                    in0=ps0[:],
                                    in1=buf1[:],
                                    s0=NEG_SEED,
                                    accum_out=racc[:, col : col + 1],
                                )
                                col += 1

                # ---- finale: d2min = -2*min(racc,0); dist = sqrt(d2min+1e-12)
                # (2 Heron steps refine ScalarE's spline sqrt); sum everything.
                ncols = col
                u = fin.tile([128, ncols], F32, tag="u")
                nc.vector.tensor_scalar_min(u[:], racc[:], 0.0)
                x = fin.tile([128, ncols], F32, tag="x")
                nc.vector.tensor_scalar(x[:], u[:], -2.0, 1e-12, ALU.mult, ALU.add)
                s0t = fin.tile([128, ncols], F32, tag="s0t")
                nc.scalar.sqrt(s0t[:], x[:])
                st = s0t
                for _ in range(2):
                    r = fin.tile([128, ncols], F32, tag="r")
                    nc.vector.reciprocal(r[:], st[:])
                    t = fin.tile([128, ncols], F32, tag="t")
                    nc.vector.tensor_mul(t[:], x[:], r[:])
                    v = fin.tile([128, ncols], F32, tag="v")
                    nc.vector.tensor_add(v[:], st[:], t[:])
                    s2 = fin.tile([128, ncols], F32, tag="s2")
                    nc.vector.tensor_scalar_mul(s2[:], v[:], 0.5)
                    st = s2
                z = fin.tile([128, 1], F32, tag="z")
                nc.vector.reduce_sum(z[:], st[:], axis=AX.X)
                ones = fin.tile([128, 1], F32, tag="ones")
                nc.vector.memset(ones[:], 1.0)
                pss = pssp.tile([1, 1], F32, tag="pss")
                nc.tensor.matmul(pss[:], z[:], ones[:], start=True, stop=True)
                ob = fin.tile([1, 1], F32, tag="ob")
                nc.scalar.copy(ob[:], pss[:])
                nc.sync.dma_start(out_ap[:], ob[:])

            if repeat == 1:
                body()
            else:
                with tc.For_i(0, repeat, 1) as _i:
                    body(_i)
    return nc


_CACHE = {}


def _get_compiled(repeat=1):
    if repeat not in _CACHE:
        nc = bacc.Bacc(
            "TRN2", target_bir_lowering=False, debug=False, num_devices=N_CORES
        )
        build_kernel(nc, repeat=repeat)
        nc.compile()
        _CACHE[repeat] = nc
    return _CACHE[repeat]


def kernel(p, q):
    """Full-input chamfer loss; shards batch dim over 8 NeuronCores."""
    from concourse.bass_utils import run_bass_kernel_spmd

    p = np.asarray(p, dtype=np.float32)
    q = np.asarray(q, dtype=np.float32)
    assert p.shape == (CH, N_CORES * BPC, N, 4) and q.shape == p.shape

    nc = _get_compiled(repeat=1)
    in_maps = [
        {
            "p": np.ascontiguousarray(p[:, k * BPC : (k + 1) * BPC]),
            "q": np.ascontiguousarray(q[:, k * BPC : (k + 1) * BPC]),
        }
        for k in range(N_CORES)
    ]
    res = run_bass_kernel_spmd(nc, in_maps, list(range(N_CORES)))
    total = np.float32(0.0)
    for k in range(N_CORES):
        total += np.float32(res.results[k]["out"].reshape(()))
    return np.asarray(total, dtype=np.float32).reshape(())


# revision 7
# speedup vs baseline: 1.2548x; 1.2548x over previous
"""Chamfer loss (p3 variant) on 8 Trainium2 NeuronCores.

Computes, for p, q of shape (2, 64, 1024, 4) fp32:
    d2[c,b,n,m] = ||p3[c,b,n] - q3[c,b,m]||^2   (p3 = spatial comps 1:4)
    loss = sum(min_m sqrt(max(d2,0)+1e-12)) + sum(min_n sqrt(...))

Strategy (data-parallel over batch, 8 batches per core):
  - e[n,m] = p3.q3' - 0.5|p3|^2 - 0.5|q3'|^2 = -d2/2, produced directly in
    PSUM by a K=5 matmul over the embedding rows
       lhsT = [x, y, z, -0.5*nrm, 1],  rhs = [x', y', z', 1, -0.5*nrm'].
  - row-min of d2 == -2 * row-max of e. A custom fused DVE op
    (MAXPAIR_REDUCE: out = max(in0,in1), accum = max-reduce) consumes two
    512-wide PSUM/SBUF halves per instruction, so every d2 element crosses
    the DVE at 2 elems/cycle. ScalarE copies one half PSUM->SBUF to enable
    the dual-port read.
  - both passes (p-major row-min and q-major col-min) run as independent
    matmul phases; sqrt (+2 Heron refinements) and the final sum happen on
    a [128, 256] tile of per-chunk minima.
"""

import os
import sys

sys.path.insert(0, "/opt/trn_rl_repo")

from contextlib import ExitStack

import numpy as np

import concourse.bass as bass
import concourse.tile as tile
from concourse import bacc, mybir

# --------------------------------------------------------------------------
# Custom DVE op: out = max(in0, in1); accum_out = max(s0, max_k out[:, k])
# Registered by appending to concourse.dve_ops.OPS (see
# trainium-docs/custom-instructions/04-custom-dve-api.md).
# --------------------------------------------------------------------------
import concourse.dve_ops as dve_ops
from concourse.dve_ops import DveOp
from concourse.dve_spec import C0, Spec, Src0, Src1, lower as dve_lower, maxx
from concourse.dve_uop import DveOpSpec


def _ref_maxpair_reduce(in0, in1, c0, c1, c2):
    b = np.maximum(in0.astype(np.float32), in1.astype(np.float32))
    P = b.shape[0]
    acc = np.maximum(
        np.broadcast_to(np.asarray(c0, np.float32), (P, 1)),
        b.reshape(P, -1).max(axis=-1, keepdims=True),
    ).astype(np.float32)
    return b, acc


def _register_maxpair():
    spec = Spec(
        body=maxx(Src0, Src1),
        accum=maxx,
        accum_init=C0,
        reference=_ref_maxpair_reduce,
    )
    shas = {}
    for ver in ("v3", "v4"):
        uops = dve_lower(spec, ver=ver)
        shas[ver] = DveOpSpec(
            name="MAXPAIR_REDUCE", opcode=0, uops=uops, rd1_en=True
        ).sha(ver)
    op = DveOp("MAXPAIR_REDUCE", spec, subdim=False, uops_sha=shas)
    if all(o.name != op.name for o in dve_ops.OPS):
        dve_ops.OPS.append(op)
        dve_ops.CUSTOM_DVE_SPECS[op.name] = spec
        dve_ops._SUB_OPCODE_FOR_NAME[op.name] = (
            max(dve_ops._SUB_OPCODE_FOR_NAME.values()) + 1
        )
        assert dve_ops._SUB_OPCODE_FOR_NAME[op.name] < 0x20
    return op


MAXPAIR_REDUCE = _register_maxpair()

# --------------------------------------------------------------------------
# Kernel build
# --------------------------------------------------------------------------
N_CORES = 8
CH = 2  # complex channels
BPC = 8  # batches per core (64 / 8 cores)
N = 1024  # points per set
NCHUNK = N // 128  # partition chunks per batch
F32 = mybir.dt.float32
NEG_SEED = -3.0e38
AX = mybir.AxisListType
ALU = mybir.AluOpType

# matmul operand dtype: "f32" (exact), "bf16" (diagnostic), "f32r" (reduced)
MM_DTYPE = os.environ.get("K_MM_DTYPE", "f32")
_MM_DT = {
    "f32": F32,
    "bf16": mybir.dt.bfloat16,
    "f32r": mybir.dt.float32r,
}[MM_DTYPE]


def build_kernel(nc, repeat=1):
    p_ap = nc.dram_tensor("p", [CH, BPC, N, 4], F32, kind="ExternalInput").ap()
    q_ap = nc.dram_tensor("q", [CH, BPC, N, 4], F32, kind="ExternalInput").ap()
    out_ap = nc.dram_tensor("out", [1, 1], F32, kind="ExternalOutput").ap()
    inp = [p_ap, q_ap]

    with tile.TileContext(nc) as tc:
        with ExitStack() as ctx:
            dramp = ctx.enter_context(tc.tile_pool(name="dram", bufs=1, space="DRAM"))
            nat = ctx.enter_context(tc.tile_pool(name="nat", bufs=2))
            nrm = ctx.enter_context(tc.tile_pool(name="nrm", bufs=2))
            emb = ctx.enter_context(
                tc.tile_pool(name="emb", bufs=2 if MM_DTYPE == "f32" else 1)
            )
            psp = ctx.enter_context(tc.tile_pool(name="psp", bufs=6, space="PSUM"))
            in1p = ctx.enter_context(tc.tile_pool(name="in1p", bufs=4))
            scr = ctx.enter_context(tc.tile_pool(name="scr", bufs=3))
            fin = ctx.enter_context(tc.tile_pool(name="fin", bufs=1))
            pssp = ctx.enter_context(tc.tile_pool(name="pssp", bufs=1, space="PSUM"))

            def body(_iv=None):
                # ---- norm rows: -0.5*|x|^2 per point, staged to DRAM in
                # flat-n order so they can be DMA'd into embedding row 3/4.
                srows = {}
                for s in range(2):
                    for c in range(CH):
                        pn = nat.tile([128, 256], F32, tag="pn")
                        nc.sync.dma_start(
                            pn[:],
                            inp[s][c].rearrange("b (x u) k -> (b x) (u k)", x=16),
                        )
                        sq = nat.tile([128, 256], F32, tag="sq")
                        nc.scalar.square(sq[:], pn[:])
                        nr = nrm.tile([128, 64], F32, tag="nr")
                        nc.vector.reduce_sum(
                            nr[:],
                            sq[:].rearrange("p (u k) -> p u k", k=4)[:, :, 1:4],
                            axis=AX.X,
                        )
                        nc.vector.tensor_scalar_mul(nr[:], nr[:], -0.5)
                        srow = dramp.tile([128, 64], F32, tag=f"srow{s}{c}")
                        nc.sync.dma_start(srow[:], nr[:])
                        srows[(s, c)] = srow

                # ---- accumulator of per-chunk maxima of e = -d2/2
                racc = fin.tile([128, 4 * BPC * NCHUNK], F32, tag="racc")

                # ones row staged at partition 0 (DVE ops cannot start at
                # partition 3/4; DMA can write there)
                ones_row = fin.tile([1, BPC * N], F32, tag="ones_row")
                nc.vector.memset(ones_row[:], 1.0)

                col = 0
                for pass_ in range(2):
                    ls, rs = (0, 1) if pass_ == 0 else (1, 0)
                    for c in range(CH):
                        L = emb.tile([5, BPC * N], F32, tag="L")
                        R = emb.tile([5, BPC * N], F32, tag="R")
                        for k in range(3):
                            nc.sync.dma_start(
                                L[k : k + 1, :],
                                inp[ls][c][:, :, k + 1].rearrange("b n -> (b n)")[
                                    None, :
                                ],
                            )
                            nc.sync.dma_start(
                                R[k : k + 1, :],
                                inp[rs][c][:, :, k + 1].rearrange("b n -> (b n)")[
                                    None, :
                                ],
                            )
                        nc.sync.dma_start(L[4:5, :], ones_row[:])
                        nc.sync.dma_start(R[3:4, :], ones_row[:])
                        nc.sync.dma_start(
                            L[3:4, :],
                            srows[(ls, c)][:].rearrange("p u -> (p u)")[None, :],
                        )
                        nc.sync.dma_start(
                            R[4:5, :],
                            srows[(rs, c)][:].rearrange("p u -> (p u)")[None, :],
                        )
                        if MM_DTYPE != "f32":
                            Lm = emb.tile([5, BPC * N], _MM_DT, tag="Lm")
                            nc.vector.tensor_copy(Lm[:], L[:])
                            Rm = emb.tile([5, BPC * N], _MM_DT, tag="Rm")
                            nc.vector.tensor_copy(Rm[:], R[:])
                            L, R = Lm, Rm
                        for b in range(BPC):
                            for i in range(NCHUNK):
                                lsl = L[:, b * N + i * 128 : b * N + (i + 1) * 128]
                                ps0 = psp.tile([128, 512], F32, tag="ps")
                                nc.tensor.matmul(
                                    ps0[:],
                                    lsl,
                                    R[:, b * N : b * N + 512],
                                    start=True,
                                    stop=True,
                                )
                                ps1 = psp.tile([128, 512], F32, tag="ps")
                                nc.tensor.matmul(
                                    ps1[:],
                                    lsl,
                                    R[:, b * N + 512 : b * N + 1024],
                                    start=True,
                                    stop=True,
                                )
                                buf1 = in1p.tile([128, 512], F32, tag="b1")
                                nc.scalar.copy(buf1[:], ps1[:])
                                sc = scr.tile([128, 512], F32, tag="sc")
                                nc.vector._custom_dve(
                                    MAXPAIR_REDUCE,
                                    out=sc[:],
                                    in0=ps0[:],
                                    in1=buf1[:],
                                    s0=NEG_SEED,
                                    accum_out=racc[:, col : col + 1],
                                )
                                col += 1

                # ---- finale: d2min = -2*min(racc,0); dist = sqrt(d2min+1e-12)
                # (2 Heron steps refine ScalarE's spline sqrt); sum everything.
                ncols = col
                u = fin.tile([128, ncols], F32, tag="u")
                nc.vector.tensor_scalar_min(u[:], racc[:], 0.0)
                x = fin.tile([128, ncols], F32, tag="x")
                nc.vector.tensor_scalar(x[:], u[:], -2.0, 1e-12, ALU.mult, ALU.add)
                s0t = fin.tile([128, ncols], F32, tag="s0t")
                nc.scalar.sqrt(s0t[:], x[:])
                st = s0t
                for _ in range(2):
                    r = fin.tile([128, ncols], F32, tag="r")
                    nc.vector.reciprocal(r[:], st[:])
                    t = fin.tile([128, ncols], F32, tag="t")
                    nc.vector.tensor_mul(t[:], x[:], r[:])
                    v = fin.tile([128, ncols], F32, tag="v")
                    nc.vector.tensor_add(v[:], st[:], t[:])
                    s2 = fin.tile([128, ncols], F32, tag="s2")
                    nc.vector.tensor_scalar_mul(s2[:], v[:], 0.5)
                    st = s2
                z = fin.tile([128, 1], F32, tag="z")
                nc.vector.reduce_sum(z[:], st[:], axis=AX.X)
                ones = fin.tile([128, 1], F32, tag="ones")
                nc.vector.memset(ones[:], 1.0)
                pss = pssp.tile([1, 1], F32, tag="pss")
                nc.tensor.matmul(pss[:], z[:], ones[:], start=True, stop=True)
                ob = fin.tile([1, 1], F32, tag="ob")
                nc.scalar.copy(ob[:], pss[:])
                nc.sync.dma_start(out_ap[:], ob[:])

            if repeat == 1:
                body()
            else:
                with tc.For_i(0, repeat, 1) as _i:
                    body(_i)
    return nc


_CACHE = {}


def _get_compiled(repeat=1):
    if repeat not in _CACHE:
        nc = bacc.Bacc(
            "TRN2", target_bir_lowering=False, debug=False, num_devices=N_CORES
        )
        build_kernel(nc, repeat=repeat)
        nc.compile()
        _CACHE[repeat] = nc
    return _CACHE[repeat]


def kernel(p, q):
    """Full-input chamfer loss; shards batch dim over 8 NeuronCores."""
    from concourse.bass_utils import run_bass_kernel_spmd

    p = np.asarray(p, dtype=np.float32)
    q = np.asarray(q, dtype=np.float32)
    assert p.shape == (CH, N_CORES * BPC, N, 4) and q.shape == p.shape

    nc = _get_compiled(repeat=1)
    in_maps = [
        {
            "p": np.ascontiguousarray(p[:, k * BPC : (k + 1) * BPC]),
            "q": np.ascontiguousarray(q[:, k * BPC : (k + 1) * BPC]),
        }
        for k in range(N_CORES)
    ]
    res = run_bass_kernel_spmd(nc, in_maps, list(range(N_CORES)))
    total = np.float32(0.0)
    for k in range(N_CORES):
        total += np.float32(res.results[k]["out"].reshape(()))
    return np.asarray(total, dtype=np.float32).reshape(())
